# revision 7
# baseline (speedup 1.0000x reference)
"""Trainium2 Bass kernel for CachedMultiHeadAttention (B=4, S=2048, D=1024, H=16).

Returns (output [B,S,D] f32, attn [B,H,S,S] f32) matching the jax reference.

Sharding: 8 cores = (batch b in 0..3) x (head-group hg in 0..1).
Each core computes 1 batch x 8 heads end-to-end:
  QT/KT = (x @ w.T).T via pre-transposed host inputs, V likewise,
  S = QK^T/8 (f32r matmuls), masked exp via (S+1000)*m then exp(x/8-125),
  row-normalized attn written to HBM (upper/masked-out tiles skipped --
  output buffers are pre-zeroed), then a transposed recompute of exp(S^T)
  feeds O^T = V^T E^T with a ones-column giving row sums, and the output
  projection Y = O @ wo_p^T. Host sums the two head-group partials and adds
  bo + bv @ wo_p^T (the V-bias commutes through softmax since rows of A sum
  to 1).
"""

import numpy as np

_B, _S, _D = 4, 2048, 1024
_H, _HPC, _DK = 16, 8, 64
_NB = _S // 128    # 16 i/j blocks of 128
_NCH = _S // 512   # 4 chunks of 512
_ZERO, _MIXED, _ONE = 0, 1, 2

_cache = {}


def _classify(mask_b):
    """mask_b: bool [B, S, S]. Tile classes shared by all cores (one program).

    ONE only if all batches all-ones in the 128x128 tile; ZERO only if all
    batches all-zeros; else MIXED (per-core mask data applied at runtime).
    """
    m_and = mask_b.all(axis=0)
    m_or = mask_b.any(axis=0)
    tclass = np.empty((_NB, _NB), np.int8)
    for i in range(_NB):
        for j in range(_NB):
            sa = m_and[i * 128:(i + 1) * 128, j * 128:(j + 1) * 128]
            so = m_or[i * 128:(i + 1) * 128, j * 128:(j + 1) * 128]
            if sa.all():
                tclass[i, j] = _ONE
            elif not so.any():
                tclass[i, j] = _ZERO
            else:
                tclass[i, j] = _MIXED
    return tclass


def _plan(tclass):
    cclass = np.empty((_NB, _NCH), np.int8)
    for i in range(_NB):
        for jc in range(_NCH):
            sub = tclass[i, jc * 4:(jc + 1) * 4]
            if (sub == _ZERO).all():
                cclass[i, jc] = _ZERO
            elif (sub == _ONE).all():
                cclass[i, jc] = _ONE
            else:
                cclass[i, jc] = _MIXED
    jlast = np.zeros(_NB, np.int64)
    for i in range(_NB):
        nz = np.nonzero(cclass[i] != _ZERO)[0]
        jlast[i] = (nz[-1] + 1) if len(nz) else 0
    st_units = []
    for ci in range(_NCH):
        units = [jt for jt in range(_NB)
                 if any(tclass[ci * 4 + ib, jt] != _ZERO for ib in range(4))]
        st_units.append(units)
    # exp groups: pairs of 512-chunks (one ACT instruction per group); a group
    # is ONE only if every chunk in it is ONE-class, else masked via mask data.
    groups = []
    wlast = np.zeros(_NB, np.int64)
    for i in range(_NB):
        jl = int(jlast[i])
        gs = []
        jc = 0
        while jc < jl:
            n = 2 if jc + 1 < jl else 1
            mixed = any(cclass[i][jc + t] != _ONE for t in range(n))
            gs.append((jc, n, mixed))
            jc += n
        groups.append(gs)
        nz = np.nonzero(tclass[i] != _ZERO)[0]
        wlast[i] = 128 * (nz[-1] + 1) if len(nz) else 0
    return cclass, jlast, st_units, groups, wlast


def _build(tclass_key):
    import concourse.mybir as mybir
    import concourse.tile as tile
    from concourse import bacc

    tclass = np.frombuffer(tclass_key, np.int8).reshape(_NB, _NB)
    cclass, jlast, st_units, groups, wlast = _plan(tclass)
    mf_bufs = [max(1, max(sum(1 for g in groups[ci * 4 + ib] if g[2])
                          for ci in range(_NCH)))
               for ib in range(4)]
    mt_bufs = [max(1, max(sum(1 for jt in st_units[ci]
                              if tclass[ci * 4 + ib, jt] == _MIXED)
                          for ci in range(_NCH)))
               for ib in range(4)]

    f32 = mybir.dt.float32
    f32r = mybir.dt.float32r
    u8 = mybir.dt.uint8
    EXP = mybir.ActivationFunctionType.Exp
    AX = mybir.AxisListType.X
    ADD = mybir.AluOpType.add
    MULT = mybir.AluOpType.mult

    nc = bacc.Bacc(None, target_bir_lowering=False)

    xqT = nc.dram_tensor("xqT", [_D, _S], f32r, kind="ExternalInput")
    xkT = nc.dram_tensor("xkT", [_D, _S], f32r, kind="ExternalInput")
    xvT = nc.dram_tensor("xvT", [_D, _S], f32r, kind="ExternalInput")
    wqT = nc.dram_tensor("wqT", [_D, 512], f32r, kind="ExternalInput")
    wkT = nc.dram_tensor("wkT", [_D, 512], f32r, kind="ExternalInput")
    wvT = nc.dram_tensor("wvT", [_D, 512], f32r, kind="ExternalInput")
    woT = nc.dram_tensor("woT", [512, _D], f32r, kind="ExternalInput")
    bq2 = nc.dram_tensor("bq2", [128, 4], f32, kind="ExternalInput")
    ones_in = nc.dram_tensor("ones_in", [128, 64], f32r, kind="ExternalInput")
    bk2 = nc.dram_tensor("bk2", [128, 4], f32, kind="ExternalInput")
    mask_u8 = nc.dram_tensor("mask_u8", [_S, _S], u8, kind="ExternalInput")
    maskT_u8 = nc.dram_tensor("maskT_u8", [_S, _S], u8, kind="ExternalInput")
    attn_out = nc.dram_tensor("attn_out", [_HPC, _S, _S], f32, kind="ExternalOutput")
    y_out = nc.dram_tensor("y_out", [_S, _D], f32, kind="ExternalOutput")

    with tile.TileContext(nc) as tc:
        with tc.tile_pool(name="pers", bufs=1) as pers:
            QT = [pers.tile([128, _S], f32r, name=f"QT{p}") for p in range(4)]
            KT = [pers.tile([128, _S], f32r, name=f"KT{p}") for p in range(4)]
            VA = [pers.tile([128, 8 * 65], f32r, name=f"VA{j}") for j in range(_NB)]
            WO = [pers.tile([128, _D], f32r, name=f"WOt{f}") for f in range(4)]
            bq_sb = pers.tile([128, 4], f32, name="bq_sb")
            bk_sb = pers.tile([128, 4], f32, name="bk_sb")
            ones_col = pers.tile([1, 64], f32r, name="ones_col")
            neg125 = pers.tile([128, 1], f32, name="neg125")

            for f in range(4):
                nc.sync.dma_start(out=WO[f][:], in_=woT[f * 128:(f + 1) * 128, :])
            nc.sync.dma_start(out=bq_sb[:], in_=bq2[:, :])
            nc.sync.dma_start(out=bk_sb[:], in_=bk2[:, :])
            nc.sync.dma_start(out=ones_col[:], in_=ones_in[0:1, 0:64])
            nc.vector.memset(neg125[:], -125.0)
            ones_col8 = ones_in[:, 0:8].rearrange("p (a b) -> p a b", b=1)
            for j in range(_NB):
                va_ones = VA[j][:].rearrange("p (h w) -> p h w", w=65)[:, :, 64:65]
                nc.sync.dma_start(out=va_ones, in_=ones_col8)

            # ---------------- Phase 1: projections ----------------
            with tc.tile_pool(name="ph1", bufs=2) as ph1, \
                 tc.tile_pool(name="ph1ps", bufs=4, space="PSUM") as ph1ps:
                for xname, xdram, wdram, qk_out, bias_sb in (
                    ("q", xqT, wqT, QT, bq_sb),
                    ("k", xkT, wkT, KT, bk_sb),
                    ("v", xvT, wvT, None, None),
                ):
                    wts = []
                    for dt in range(8):
                        wt = ph1.tile([128, 512], f32r, name=f"wt_{xname}_{dt}",
                                      tag=f"wt{dt}")
                        nc.sync.dma_start(out=wt[:], in_=wdram[dt * 128:(dt + 1) * 128, :])
                        wts.append(wt)
                    for sc in range(4):
                        xts = []
                        for dt in range(8):
                            xt = ph1.tile([128, 512], f32r,
                                          name=f"xt_{xname}_{sc}_{dt}", tag=f"xt{dt}")
                            nc.sync.dma_start(
                                out=xt[:],
                                in_=xdram[dt * 128:(dt + 1) * 128,
                                          sc * 512:(sc + 1) * 512])
                            xts.append(xt)
                        if qk_out is not None:
                            for ct in range(4):
                                ps = ph1ps.tile([128, 512], f32,
                                                name=f"ps_{xname}_{sc}_{ct}", tag="psP")
                                for dt in range(8):
                                    nc.tensor.matmul(
                                        ps[:], wts[dt][:, ct * 128:(ct + 1) * 128],
                                        xts[dt][:],
                                        start=(dt == 0), stop=(dt == 7))
                                nc.scalar.add(
                                    qk_out[ct][:, sc * 512:(sc + 1) * 512], ps[:],
                                    add=bias_sb[:, ct:ct + 1])
                        else:
                            for jq in range(4):
                                jt = sc * 4 + jq
                                ps = ph1ps.tile([128, 512], f32,
                                                name=f"ps_v_{jt}", tag="psP")
                                for dt in range(8):
                                    nc.tensor.matmul(
                                        ps[:],
                                        xts[dt][:, jq * 128:(jq + 1) * 128],
                                        wts[dt][:],
                                        start=(dt == 0), stop=(dt == 7))
                                va_v = VA[jt][:].rearrange(
                                    "p (h w) -> p h w", w=65)[:, :, 0:64]
                                ps_v = ps[:].rearrange("p (h w) -> p h w", w=64)
                                nc.vector.tensor_copy(out=va_v, in_=ps_v)

            # ---------------- Phase 2: attention ----------------
            with tc.tile_pool(name="pE", bufs=2) as pE, \
                 tc.tile_pool(name="pET", bufs=3) as pET, \
                 tc.tile_pool(name="pM", bufs=1) as pM, \
                 tc.tile_pool(name="pSm", bufs=3) as pSm, \
                 tc.tile_pool(name="pOT", bufs=2) as pOT, \
                 tc.tile_pool(name="pY", bufs=2) as pY, \
                 tc.tile_pool(name="psS", bufs=2, space="PSUM") as psS, \
                 tc.tile_pool(name="psT", bufs=2, space="PSUM") as psT, \
                 tc.tile_pool(name="psU", bufs=1, space="PSUM") as psU:
                for ci in range(_NCH):
                    # stage mask tiles shared by all 4 head-pairs
                    mf = {}
                    for ib in range(4):
                        I = ci * 4 + ib
                        for gi, (g0, gn, gmix) in enumerate(groups[I]):
                            if not gmix:
                                continue
                            w = gn * 512
                            mu = pM.tile([128, 1024], u8, name=f"mu_{ci}_{ib}_{gi}",
                                         tag="mu8", bufs=2)
                            nc.sync.dma_start(
                                out=mu[:, 0:w],
                                in_=mask_u8[I * 128:(I + 1) * 128,
                                            g0 * 512:g0 * 512 + w])
                            t = pM.tile([128, 1024], f32, name=f"mf_{ci}_{ib}_{gi}",
                                        tag=f"mf_{ib}", bufs=mf_bufs[ib])
                            nc.vector.tensor_copy(out=t[:, 0:w], in_=mu[:, 0:w])
                            mf[(ib, gi)] = t
                    mt = {}
                    for jt in st_units[ci]:
                        for ib in range(4):
                            I = ci * 4 + ib
                            if tclass[I, jt] != _MIXED:
                                continue
                            mtu = pM.tile([128, 128], u8, name=f"mtu_{ci}_{jt}_{ib}",
                                          tag="mtu8", bufs=2)
                            nc.sync.dma_start(
                                out=mtu[:],
                                in_=maskT_u8[jt * 128:(jt + 1) * 128,
                                             I * 128:(I + 1) * 128])
                            t = pM.tile([128, 128], f32r, name=f"mt_{ci}_{jt}_{ib}",
                                        tag=f"mt_{ib}", bufs=mt_bufs[ib])
                            nc.vector.tensor_copy(out=t[:], in_=mtu[:])
                            mt[(jt, ib)] = t

                    ot_tiles = {}
                    for p in range(4):
                        # ---- natural pass: S -> E -> attn rows ----
                        for ib in range(4):
                            I = ci * 4 + ib
                            jl = int(jlast[I])
                            if jl == 0:
                                continue
                            gs = groups[I]
                            wl = int(wlast[I])
                            for s2 in range(2):
                                h = 2 * p + s2
                                E = pE.tile([128, _S], f32,
                                            name=f"E_{ci}_{p}_{ib}_{s2}", tag="E")
                                rp = pSm.tile([128, 2], f32,
                                              name=f"rp_{ci}_{p}_{ib}_{s2}", tag="rp")
                                for gi, (g0, gn, gmix) in enumerate(gs):
                                    w = gn * 512
                                    ps = psS.tile([128, 1024], f32,
                                                  name=f"psS_{ci}_{p}_{ib}_{s2}_{gi}",
                                                  tag="psS")
                                    for t in range(gn):
                                        nc.tensor.matmul(
                                            ps[:, t * 512:(t + 1) * 512],
                                            QT[p][64 * s2:64 * s2 + 64,
                                                  I * 128:(I + 1) * 128],
                                            KT[p][64 * s2:64 * s2 + 64,
                                                  (g0 + t) * 512:(g0 + t + 1) * 512],
                                            start=True, stop=True)
                                    if gmix:
                                        nc.vector.scalar_tensor_tensor(
                                            out=ps[:, 0:w], in0=ps[:, 0:w],
                                            scalar=1000.0,
                                            in1=mf[(ib, gi)][:, 0:w],
                                            op0=ADD, op1=MULT)
                                        bias = neg125[:]
                                    else:
                                        bias = 0.0
                                    nc.scalar.activation(
                                        E[:, g0 * 512:g0 * 512 + w], ps[:, 0:w], EXP,
                                        bias=bias, scale=0.125,
                                        accum_out=rp[:, gi:gi + 1])
                                r = pSm.tile([128, 1], f32,
                                             name=f"r_{ci}_{p}_{ib}_{s2}", tag="r")
                                rv = pSm.tile([128, 1], f32,
                                              name=f"rv_{ci}_{p}_{ib}_{s2}", tag="rv")
                                nc.vector.reduce_sum(out=r[:], in_=rp[:, 0:len(gs)],
                                                     axis=AX)
                                nc.vector.tensor_scalar_add(
                                    out=r[:], in0=r[:], scalar1=1e-30)
                                nc.vector.reciprocal(rv[:], r[:])
                                nc.vector.tensor_scalar_mul(
                                    out=E[:, 0:wl], in0=E[:, 0:wl],
                                    scalar1=rv[:])
                                nc.sync.dma_start(
                                    out=attn_out[h, I * 128:(I + 1) * 128, 0:wl],
                                    in_=E[:, 0:wl])

                        # ---- transposed pass: E^T, U^T = V^T E^T, row sums ----
                        units = st_units[ci]
                        ot = pOT.tile([128, 512], f32r, name=f"ot_{ci}_{p}",
                                      tag=f"ot{p}")
                        if not units:
                            oz = ones_in[:, 0:64].rearrange(
                                "p (a b) -> p (a b)", b=64)
                            nc.sync.dma_start(out=ot[:, 0:64], in_=oz)
                            nc.sync.dma_start(out=ot[:, 64:128], in_=oz)
                            for zz in range(2, 8):
                                nc.sync.dma_start(
                                    out=ot[:, zz * 64:(zz + 1) * 64], in_=oz)
                            nc.vector.tensor_scalar_mul(
                                out=ot[:], in0=ot[:], scalar1=0.0)
                            ot_tiles[p] = ot
                            continue
                        pus = [psU.tile([65, 512], f32,
                                        name=f"psU_{ci}_{p}_{s2}", tag=f"psU{s2}",
                                        bufs=1)
                               for s2 in range(2)]
                        for u, jt in enumerate(units):
                            for s2 in range(2):
                                h = 2 * p + s2
                                pst = psT.tile([128, 512], f32,
                                               name=f"psT_{ci}_{p}_{jt}_{s2}",
                                               tag="psT")
                                nc.tensor.matmul(
                                    pst[:],
                                    KT[p][64 * s2:64 * s2 + 64,
                                          jt * 128:(jt + 1) * 128],
                                    QT[p][64 * s2:64 * s2 + 64,
                                          ci * 512:(ci + 1) * 512],
                                    start=True, stop=True)
                                et = pET.tile([128, 512], f32r,
                                              name=f"et_{ci}_{p}_{jt}_{s2}", tag="ET")
                                nc.scalar.activation(et[:], pst[:], EXP, scale=0.125)
                                for ib in range(4):
                                    c = tclass[ci * 4 + ib, jt]
                                    if c == _ZERO:
                                        nc.vector.tensor_scalar_mul(
                                            out=et[:, ib * 128:(ib + 1) * 128],
                                            in0=et[:, ib * 128:(ib + 1) * 128],
                                            scalar1=0.0)
                                    elif c == _MIXED:
                                        nc.vector.tensor_mul(
                                            out=et[:, ib * 128:(ib + 1) * 128],
                                            in0=et[:, ib * 128:(ib + 1) * 128],
                                            in1=mt[(jt, ib)][:])
                                nc.tensor.matmul(
                                    pus[s2][:],
                                    VA[jt][:, h * 65:(h + 1) * 65],
                                    et[:],
                                    start=(u == 0), stop=(u == len(units) - 1),
                                    skip_group_check=True)
                        for s2 in range(2):
                            rr_a = pSm.tile([1, 512], f32,
                                            name=f"rra_{ci}_{p}_{s2}", tag="rra")
                            rr_b = pSm.tile([1, 512], f32r,
                                            name=f"rrb_{ci}_{p}_{s2}", tag="rrb")
                            nc.vector.tensor_scalar_add(
                                out=rr_a[:], in0=pus[s2][64:65, :], scalar1=1e-30)
                            with nc.allow_low_precision(reason="f32r rinv"):
                                nc.vector.reciprocal(rr_b[:], rr_a[:])
                            psb = psT.tile([64, 512], f32,
                                           name=f"psb_{ci}_{p}_{s2}", tag="psT")
                            nc.tensor.matmul(psb[:], ones_col[:], rr_b[:],
                                             start=True, stop=True)
                            rb_sb = pSm.tile([64, 512], f32,
                                             name=f"rb_{ci}_{p}_{s2}", tag="rb")
                            nc.vector.tensor_copy(out=rb_sb[:], in_=psb[:])
                            nc.vector.tensor_mul(
                                out=ot[64 * s2:64 * s2 + 64, :],
                                in0=pus[s2][0:64, :], in1=rb_sb[:])
                        ot_tiles[p] = ot

                    # ---- output projection for this i-chunk ----
                    for isub in range(4):
                        y = pY.tile([128, _D], f32, name=f"y_{ci}_{isub}", tag="y")
                        for ec in range(2):
                            psy = psT.tile([128, 512], f32,
                                           name=f"psy_{ci}_{isub}_{ec}", tag="psT")
                            for ft in range(4):
                                nc.tensor.matmul(
                                    psy[:],
                                    ot_tiles[ft][:, isub * 128:(isub + 1) * 128],
                                    WO[ft][:, ec * 512:(ec + 1) * 512],
                                    start=(ft == 0), stop=(ft == 3),
                                    skip_group_check=True)
                            nc.vector.tensor_copy(
                                out=y[:, ec * 512:(ec + 1) * 512], in_=psy[:])
                        nc.sync.dma_start(
                            out=y_out[(ci * 4 + isub) * 128:(ci * 4 + isub + 1) * 128, :],
                            in_=y[:])

    nc.compile()
    return nc


def _prep_in_maps(inputs):
    q = np.ascontiguousarray(np.asarray(inputs["query"], np.float32))
    k = np.ascontiguousarray(np.asarray(inputs["key"], np.float32))
    v = np.ascontiguousarray(np.asarray(inputs["value"], np.float32))
    mask = np.asarray(inputs["mask"]).astype(bool).reshape(_B, _S, _S)
    wq = np.asarray(inputs["wq"], np.float32)
    wk = np.asarray(inputs["wk"], np.float32)
    wv = np.asarray(inputs["wv"], np.float32)
    wo = np.asarray(inputs["wo"], np.float32)
    bq = np.asarray(inputs["bq"], np.float32)
    bk = np.asarray(inputs["bk"], np.float32)

    in_maps = []
    for c in range(8):
        b, hg = c // 2, c % 2
        r0, r1 = hg * 512, (hg + 1) * 512
        m_u8 = np.ascontiguousarray(mask[b].astype(np.uint8))
        mT_u8 = np.ascontiguousarray(mask[b].T.astype(np.uint8))
        in_maps.append({
            "xqT": np.ascontiguousarray(q[b].T),
            "xkT": np.ascontiguousarray(k[b].T),
            "xvT": np.ascontiguousarray(v[b].T),
            "wqT": np.ascontiguousarray(wq[r0:r1, :].T),
            "wkT": np.ascontiguousarray(wk[r0:r1, :].T),
            "wvT": np.ascontiguousarray(wv[r0:r1, :].T),
            "woT": np.ascontiguousarray(wo[:, r0:r1].T),
            "bq2": np.ascontiguousarray(bq[r0:r1].reshape(4, 128).T),
            "bk2": np.ascontiguousarray(bk[r0:r1].reshape(4, 128).T),
            "ones_in": np.ones((128, 64), np.float32),
            "mask_u8": m_u8,
            "maskT_u8": mT_u8,
        })
    return in_maps, mask


def _get_nc(mask):
    key = _classify(mask).tobytes()
    if key not in _cache:
        _cache[key] = _build(key)
    return _cache[key]


def kernel(**inputs):
    from concourse.bass_utils import run_bass_kernel_spmd

    in_maps, mask = _prep_in_maps(inputs)
    nc = _get_nc(mask)
    res = run_bass_kernel_spmd(nc, in_maps, core_ids=list(range(8)))

    wo = np.asarray(inputs["wo"], np.float32)
    bv = np.asarray(inputs["bv"], np.float32)
    bo = np.asarray(inputs["bo"], np.float32)

    attn = np.empty((_B, _H, _S, _S), np.float32)
    out = np.zeros((_B, _S, _D), np.float32)
    for c in range(8):
        b, hg = c // 2, c % 2
        attn[b, hg * 8:(hg + 1) * 8] = res.results[c]["attn_out"]
        out[b] += res.results[c]["y_out"]
    # bias folds: O = A(V + bv) = AV + bv (rows of A sum to 1), so
    # Y += bv @ wo^T; plus the output bias bo.
    out += (bv @ wo.T + bo).astype(np.float32)
    return out, attn
